# revision 34
# baseline (speedup 1.0000x reference)
"""Trainium2 Bass kernel for nn_LowRankLayer_dilation (B=4, C=64, H=W=128).

Math: the reference's rank-3 NMF update collapses exactly (all ranks are
initialized identically), and the eps terms are negligible for this input
distribution, giving:

    h   = relu(W_head @ x)            (per-pixel channel matmul)
    g   = W_tail @ h                  (per-pixel channel matmul)
    a   = box9(h)                     (3x3 dilation-2 box sum, edge-clamped)
    n_k = sum_c (a/9)_c * h_c(p+d_k)  (9 taps, d in {-2,0,2}^2)
    out = x + (n_4 / sum_j n_j^2) * sum_k n_k * g(p+d_k)

Sharding: pure data parallel, 8 cores = (batch b, H-half). Each core gets a
68-row halo'd slice packed as 2 channel blocks on 128 partitions:
partition p = c + 64*blk, blk A = slice rows 0..35, blk B = rows 32..67.
Channel reductions/broadcasts run on the PE via block-structured 0/1
matrices. h and g keep 2 replicate-padded columns per side (row stride 132)
so every dilated tap is a strided AP view.

v2 engine assignment (vs v1): the 9 taps are processed in 3 row-groups of 3,
each group's products computed by ONE 4-D-AP DVE op (the dj axis is an
overlapping stride-2 AP dim; av is replicated via a stride-0 broadcast dim).
PSUM->SBUF broadcast copies are split Scalar/GpSimd per 1024-chunk; the box
filter is split DVE/GpSimd; relu runs as tensor_max against a broadcast zero
tile (2x DVE mode) or on Scalar; the residual add reads the bf16 input tile
directly (no separate f32 residual DMA) and the output is bf16. All small
gather/output DMAs issue from the idle sync (SP) queue to keep GpSimd free.
"""
import sys
import contextlib
import numpy as np

sys.path.insert(0, '/opt/trn_rl_repo')

import concourse.bass as bass  # noqa: E402,F401
import concourse.bacc as bacc  # noqa: E402
import concourse.tile as tile  # noqa: E402
import concourse.mybir as mybir  # noqa: E402
from concourse.bass_utils import run_bass_kernel_spmd  # noqa: E402

F32 = mybir.dt.float32
BF16 = mybir.dt.bfloat16
AT = mybir.ActivationFunctionType

N_CORES = 8
RIN = 36          # per-block input rows (with +-2 halo)
ROUT = 32         # per-block output rows
W = 128
WP = W + 4        # padded row stride for h/g
FIN = RIN * W     # 4608
FOUT = ROUT * W   # 4096
HF = 2048         # half (16 out rows) worth of pixels per partition

EDT = BF16


def _build():
    nc = bacc.Bacc("TRN2", target_bir_lowering=False, debug=False,
                   num_devices=N_CORES)
    xb_ext = nc.dram_tensor("xb", [128, FIN], EDT, kind="ExternalInput").ap()
    w2_ext = nc.dram_tensor("w2", [128, 128], EDT, kind="ExternalInput").ap()
    w3_ext = nc.dram_tensor("w3", [128, 128], EDT, kind="ExternalInput").ap()
    bo_ext = nc.dram_tensor("bo", [128, 128], EDT, kind="ExternalInput").ap()
    sb_ext = nc.dram_tensor("sb", [18, 128], EDT, kind="ExternalInput").ap()
    y_ext = nc.dram_tensor("y", [128, FOUT], EDT, kind="ExternalOutput").ap()

    with tile.TileContext(nc) as tc, contextlib.ExitStack() as ctx:
        cpool = ctx.enter_context(tc.tile_pool(name="consts", bufs=1))
        big = ctx.enter_context(tc.tile_pool(name="big", bufs=1))
        gpool = ctx.enter_context(tc.tile_pool(name="grp", bufs=2))
        rows = ctx.enter_context(tc.tile_pool(name="rows", bufs=2))

        # startup: weights first (gate LDW), x chunks spread over the three
        # HWDGE queues so transfers run in parallel
        w2 = cpool.tile([128, 128], EDT)
        nc.gpsimd.dma_start(w2[:], w2_ext[:])
        xbt = big.tile([128, FIN], EDT)
        nc.gpsimd.dma_start(xbt[:, 0:512], xb_ext[:, 0:512])
        nc.sync.dma_start(xbt[:, 512:2048], xb_ext[:, 512:2048])
        nc.scalar.dma_start(xbt[:, 2048:4096], xb_ext[:, 2048:4096])
        nc.gpsimd.dma_start(xbt[:, 4096:4608], xb_ext[:, 4096:4608])
        w3 = cpool.tile([128, 128], EDT)
        nc.sync.dma_start(w3[:], w3_ext[:])
        bo = cpool.tile([128, 128], EDT)
        nc.gpsimd.dma_start(bo[:], bo_ext[:])
        sbm = cpool.tile([18, 128], EDT)
        nc.gpsimd.dma_start(sbm[:], sb_ext[:])

        # h/g: (RIN, WP) row layout; data at cols 2..129, replicate pads at
        # cols 0,1,130,131.
        hf = big.tile([128, RIN * WP], EDT)
        h3 = hf.rearrange("p (r w) -> p r w", w=WP)

        def tap(t3, di, dj, nrows=ROUT, r0=2):
            rr = r0 + di
            return t3[:, rr:rr + nrows, 2 + dj:2 + dj + W]

        PADS = ((0, 2), (1, 2), (130, 129), (131, 129))

        # ---- head + tail matmuls: h = relu(W_head @ x), g = W_tail @ h ----
        with tc.tile_pool(name="psmm", bufs=2, space="PSUM") as psmm:
            for j in range(2):
                ps = psmm.tile([128, 2048], F32, tag="mm")
                for q in range(4):
                    c0 = j * 2048 + q * 512
                    nc.tensor.matmul(ps[:, q * 512:(q + 1) * 512], w2[:],
                                     xbt[:, c0:c0 + 512], start=True, stop=True)
                if j == 0:
                    for hh in range(2):
                        nc.vector.tensor_scalar_max(
                            h3[:, hh * 8:hh * 8 + 8, 2:2 + W],
                            ps[:, hh * 1024:(hh + 1) * 1024]
                            .rearrange("p (r w) -> p r w", w=W), 0.0)
                else:
                    nc.scalar.activation(
                        h3[:, j * 16:j * 16 + 16, 2:2 + W],
                        ps[:].rearrange("p (r w) -> p r w", w=W), AT.Relu)
            for dst_c, src_c in PADS:
                nc.scalar.copy(h3[:, 0:32, dst_c:dst_c + 1],
                               h3[:, 0:32, src_c:src_c + 1])
            ps = psmm.tile([128, 2048], F32, tag="mm")
            nc.tensor.matmul(ps[:, 0:512], w2[:], xbt[:, 4096:4608],
                             start=True, stop=True)
            nc.scalar.activation(h3[:, 32:36, 2:2 + W],
                                 ps[:, 0:512].rearrange("p (r w) -> p r w", w=W),
                                 AT.Relu)
            for dst_c, src_c in PADS:
                nc.scalar.copy(h3[:, 32:36, dst_c:dst_c + 1],
                               h3[:, 32:36, src_c:src_c + 1])

            # box filter row pass (DVE): T rows 0..19 first (gates av half 0)
            T = big.tile([128, FIN], EDT)
            T3 = T.rearrange("p (r w) -> p r w", w=W)
            nc.vector.tensor_add(T3[:, 0:20, :], tap(h3, -2, -2, 20, 2),
                                 tap(h3, -2, 0, 20, 2))
            nc.vector.tensor_add(T3[:, 0:20, :], T3[:, 0:20, :],
                                 tap(h3, -2, 2, 20, 2))
            av = big.tile([128, FOUT], EDT)
            nc.vector.tensor_add(av[:, 0:HF], T[:, 0:HF],
                                 T[:, 2 * W:2 * W + HF])
            nc.vector.tensor_add(av[:, 0:HF], av[:, 0:HF],
                                 T[:, 4 * W:4 * W + HF])

            def emit_box1():
                nc.vector.tensor_add(T3[:, 20:36, :],
                                     tap(h3, -2, -2, 16, 22),
                                     tap(h3, -2, 0, 16, 22))
                nc.vector.tensor_add(T3[:, 20:36, :], T3[:, 20:36, :],
                                     tap(h3, -2, 2, 16, 22))
                nc.vector.tensor_add(av[:, HF:2 * HF], T[:, HF:2 * HF],
                                     T[:, HF + 2 * W:HF + 2 * W + HF])
                nc.vector.tensor_add(av[:, HF:2 * HF], av[:, HF:2 * HF],
                                     T[:, HF + 4 * W:HF + 4 * W + HF])


        av3 = av.rearrange("p (r w) -> p r w", w=W)

        # ---- k loop ----
        OFFS = [(di, dj) for di in (-2, 0, 2) for dj in (-2, 0, 2)]
        nst = cpool.tile([18, FOUT], EDT)       # n_k rows, row pair by kr
        nsq = cpool.tile([18, FOUT], EDT)

        with tc.tile_pool(name="psnk", bufs=2, space="PSUM") as psnk, \
                tc.tile_pool(name="psfa", bufs=1, space="PSUM") as psfa:

            def cf_steps(half, nb_c, facc_ps):
                """Deferred tail for one half, column-chunked: N2 (broadcast
                via sbm matmul through the psnk ring), reciprocal, cf =
                nb_center*rcp (on Pool for lagged halves), res = facc*cf + x,
                DMA out. Fed one segment per k of the NEXT half; the final
                half runs 4-way chunked to shorten the serial tail."""
                hs0 = half * HF
                lag = half == 0          # lagged half: feed during next half
                nch = 2 if lag else 4
                cw = HF // nch
                nc.scalar.square(nsq[:, hs0:hs0 + HF], nst[:, hs0:hs0 + HF])
                fsb = rows.tile([128, HF], EDT, tag="fsb", bufs=2)
                nc.scalar.copy(fsb[:], facc_ps[:])
                rcp = rows.tile([128, HF], F32, tag="rcp", bufs=2)
                cfb = rows.tile([128, HF], EDT, tag="cfb", bufs=2)
                for ch in range(nch):
                    s2ps = psnk.tile([128, 1024], F32, tag="nk")
                    for q in range(max(1, cw // 512)):
                        c0 = hs0 + ch * cw + q * 512
                        nc.tensor.matmul(s2ps[:, q * 512:q * 512 + min(512, cw)],
                                         sbm[:], nsq[:, c0:c0 + min(512, cw)],
                                         start=True, stop=True)
                    cs = slice(ch * cw, (ch + 1) * cw)
                    nc.vector.reciprocal_approx_fast(rcp[:, cs],
                                                     s2ps[:, 0:cw])
                    nc.vector.tensor_mul(cfb[:, cs], nb_c[:, cs], rcp[:, cs])
                    if lag:
                        yield
                for ch in range(nch):
                    cs = slice(ch * cw, (ch + 1) * cw)
                    res = rows.tile([128, cw], EDT, tag="res", bufs=2,
                                    name="res")
                    nc.vector.tensor_mul(res[:], fsb[:, cs], cfb[:, cs])
                    nc.vector.tensor_add(
                        res[:], res[:],
                        xbt[:, 2 * W + hs0 + ch * cw:
                             2 * W + hs0 + (ch + 1) * cw])
                    nc.sync.dma_start(y_ext[:, hs0 + ch * cw:
                                            hs0 + (ch + 1) * cw], res[:])
                    if lag:
                        yield

            pending = None
            for half in range(2):
                rh = half * 16
                hs = slice(half * HF, (half + 1) * HF)
                facc_ps = psfa.tile([128, HF], F32, tag="fa")

                def emit_prod(k):
                    di, dj = OFFS[k]
                    prod = gpool.tile([128, HF], EDT, tag="pp", bufs=3)
                    p3 = prod.rearrange("p (r w) -> p r w", w=W)
                    nc.vector.tensor_mul(p3[:], av3[:, rh:rh + 16, :],
                                         tap(h3, di, dj, 16, 2 + rh))
                    return prod

                prods = {0: emit_prod(0), 1: emit_prod(1)}
                pks = {}
                nb3 = None
                nbc = [None]

                def emit_ident(k):
                    # facc += W_tail.T @ (nb*h_tap) == n_k * g(p+d_k): the
                    # tail 1x1 conv is folded into the accumulation matmul
                    pk = pks.pop(k)
                    for q in range(4):
                        nc.tensor.matmul(
                            facc_ps[:, q * 512:(q + 1) * 512], w3[:],
                            pk[:, q * 512:(q + 1) * 512],
                            start=(k == 0), stop=(k == 8))

                for k, (di, dj) in enumerate(OFFS):
                    grp, j = divmod(k, 3)
                    if j == 0:
                        nb3 = gpool.tile([128, 3 * HF], EDT, tag="nb3",
                                         bufs=3)
                        if grp == 1:
                            nbc[0] = nb3[:, HF:2 * HF]
                    prod = prods.pop(k)
                    for ch in range(2):
                        pst = psnk.tile([128, 1024], F32, tag="nk")
                        for q in range(2):
                            c0 = ch * 1024 + q * 512
                            nc.tensor.matmul(
                                pst[:, q * 512:(q + 1) * 512], bo[:],
                                prod[:, c0:c0 + 512],
                                start=True, stop=True)
                        nc.scalar.copy(
                            nb3[:, j * HF + ch * 1024:
                                j * HF + (ch + 1) * 1024], pst[:])
                    kr = (k - 4) % 9          # put k=4 (center) at rows 0..1
                    nc.sync.dma_start(nst[2 * kr:2 * kr + 1, hs],
                                      nb3[0:1, j * HF:(j + 1) * HF])
                    nc.sync.dma_start(nst[2 * kr + 1:2 * kr + 2, hs],
                                      nb3[64:65, j * HF:(j + 1) * HF])

                    if k + 2 < 9:
                        prods[k + 2] = emit_prod(k + 2)

                    pk = gpool.tile([128, HF], EDT, tag="pk", bufs=2)
                    p3 = pk.rearrange("p (r w) -> p r w", w=W)
                    nb33 = nb3[:, j * HF:(j + 1) * HF] \
                        .rearrange("p (r w) -> p r w", w=W)
                    nc.vector.tensor_mul(p3[:], nb33[:],
                                          tap(h3, di, dj, 16, 2 + rh))
                    pks[k] = pk
                    if k >= 1:
                        emit_ident(k - 1)
                    if half == 0 and k == 1:
                        emit_box1()
                    if pending is not None and k in (0, 2, 4, 6, 8):
                        next(pending, None)
                emit_ident(8)
                if pending is not None:
                    for _ in pending:
                        pass
                pending = cf_steps(half, nbc[0], facc_ps)
            for _ in pending:
                pass

    nc.compile()
    return nc


_NC_CACHE = [None]


def _get_nc():
    if _NC_CACHE[0] is None:
        _NC_CACHE[0] = _build()
    return _NC_CACHE[0]


def _host_prep(x):
    import ml_dtypes
    B, Cc, H, Ww = x.shape
    in_maps = []
    for core in range(N_CORES):
        b, half = core // 2, core % 2
        r0 = 64 * half
        gidx = np.clip(np.arange(r0 - 2, r0 + 66), 0, H - 1)
        xs = x[b][:, gidx, :]                     # (64, 68, 128)
        packed = np.ascontiguousarray(
            np.concatenate([xs[:, 0:36], xs[:, 32:68]], axis=0))
        in_maps.append({
            "xb": packed.reshape(128, FIN).astype(ml_dtypes.bfloat16),
        })
    return in_maps


def _const_maps(W_head, W_tail):
    import ml_dtypes

    def to_edt(a):
        return a.astype(ml_dtypes.bfloat16) if EDT == BF16 else a.astype(np.float32)

    w2 = np.zeros((128, 128), np.float32)
    w2[:64, :64] = W_head.T
    w2[64:, 64:] = W_head.T
    w3 = np.zeros((128, 128), np.float32)
    w3[:64, :64] = W_tail.T
    w3[64:, 64:] = W_tail.T
    bo = np.zeros((128, 128), np.float32)
    bo[:64, :64] = 1.0 / 9.0
    bo[64:, 64:] = 1.0 / 9.0
    sb = np.zeros((18, 128), np.float32)
    sb[0::2, :64] = 1.0
    sb[1::2, 64:] = 1.0
    return {"w2": to_edt(w2), "w3": to_edt(w3), "bo": to_edt(bo),
            "sb": to_edt(sb)}


def kernel(x, W_head, W_tail):
    x = np.asarray(x, np.float32)
    W_head = np.asarray(W_head, np.float32)
    W_tail = np.asarray(W_tail, np.float32)
    nc = _get_nc()
    consts = _const_maps(W_head, W_tail)
    in_maps = [{**m, **consts} for m in _host_prep(x)]
    res = run_bass_kernel_spmd(nc, in_maps, list(range(N_CORES)))
    out = np.empty_like(x)
    for core in range(N_CORES):
        b, half = core // 2, core % 2
        r0 = 64 * half
        y = res.results[core]["y"].astype(np.float32).reshape(128, ROUT, W)
        out[b, :, r0:r0 + 32, :] = y[:64]
        out[b, :, r0 + 32:r0 + 64, :] = y[64:]
    return out
